# revision 16
# baseline (speedup 1.0000x reference)
"""Multi-head attention (B=8, S=1024, D=768, H=12) on 8 TRN2 NeuronCores.

Sharding: batch-parallel — each core computes one batch item end-to-end
(weights replicated), so no collectives are needed. Host shards x, runs the
SPMD Bass kernel on cores 0-7, gathers per-core outputs.

Per-core dataflow (everything in transposed [feature, seq] layout so no
on-chip transposes are ever needed; matmul operands in fp16, accumulation in
fp32 PSUM):
  qT = (W_q/8)^T x^T + b_q/8      [768, 1024]  (1/sqrt(dk) folded in)
  kT = W_k^T x^T + b_k            [768, 1024]
  v  = x W_v + b_v                [1024, 768]  (natural; + ones column)
  per head h, q-block j (512 wide), k-tile i (128 wide, causal-skipped):
    scoresT = kT_h[:, i]^T qT_h[:, j]          [128, 512]  (PE, K=64)
    expT    = exp(scoresT)                     (ACT, PSUM->SBUF, fp16)
    expT   *= maskT pattern on mixed columns   (DVE)
    outU^T += [v_h | 1]^T expT                 (PE; row 64 = softmax denom)
  out^T = outU^T * bcast(1/rowsum)             (DVE; DRAM-bounce broadcast)
  yT = W_o^T out^T + b_o                       [768, 1024]
Host transposes yT back to [1024, 768].
"""

import numpy as np

B, S, D, H, DK = 8, 1024, 768, 12, 64
DT = D // 128        # 6  d-model tiles
ST = S // 128        # 8  seq tiles
NB = S // 512        # 2  512-wide seq blocks
HPM = 2              # heads per 128-row m-tile

_CACHE = {}


def _classify_mask(mask_bool):
    """mask_bool: [S, S] (q, k). Returns per (i, j) block descriptors for the
    scoresT layout [k, q]: list over j (512-wide q blocks) of lists over i
    (128-wide k tiles) of None (skip) or dict(exp_lo, mul, pat_id), plus the
    deduped mask patterns (each [128, w] float, transposed to [k, q])."""
    patterns = []
    pat_index = {}
    blocks = []
    for j in range(NB):
        row = []
        for i in range(ST):
            sub = mask_bool[j * 512:(j + 1) * 512, i * 128:(i + 1) * 128].T
            # sub: [k 128, q 512]
            if not sub.any():
                row.append(None)
                continue
            col_any = sub.any(axis=0)
            col_all = sub.all(axis=0)
            exp_lo = int(np.argmax(col_any))
            assert not col_any[:exp_lo].any()
            mixed = ~col_all
            mixed[:exp_lo] = False
            desc = {"exp_lo": exp_lo, "mul": None}
            if mixed.any():
                lo = int(np.argmax(mixed))
                hi = int(len(mixed) - np.argmax(mixed[::-1]))
                assert col_all[hi:].all() and col_all[exp_lo:lo].all()
                pat = sub[:, lo:hi].astype(np.float16)
                key = (pat.shape[1], pat.tobytes())
                if key not in pat_index:
                    pat_index[key] = len(patterns)
                    patterns.append(pat)
                desc["mul"] = (lo, hi)
                desc["pat_id"] = pat_index[key]
            row.append(desc)
        blocks.append(row)
    return blocks, patterns


def _build(blocks, patterns, pat_width):
    import concourse.bass as bass
    import concourse.bacc as bacc
    import concourse.mybir as mybir
    import concourse.tile as tile
    from contextlib import ExitStack

    f32 = mybir.dt.float32
    f16 = mybir.dt.float16
    AF = mybir.ActivationFunctionType

    nc = bacc.Bacc("TRN2", target_bir_lowering=False, debug=False)

    xT_d = nc.dram_tensor("xT", [D, S], f16, kind="ExternalInput").ap()
    wq_d = nc.dram_tensor("wq", [D, D], f16, kind="ExternalInput").ap()
    wk_d = nc.dram_tensor("wk", [D, D], f16, kind="ExternalInput").ap()
    wv_d = nc.dram_tensor("wv", [D, D], f16, kind="ExternalInput").ap()
    wo_d = nc.dram_tensor("wo", [D, D], f16, kind="ExternalInput").ap()
    bq_d = nc.dram_tensor("bq", [128, DT], f32, kind="ExternalInput").ap()
    bk_d = nc.dram_tensor("bk", [128, DT], f32, kind="ExternalInput").ap()
    bv_d = nc.dram_tensor("bv", [128, H, DK], f32,
                          kind="ExternalInput").ap()
    bo_d = nc.dram_tensor("bo", [128, DT], f32, kind="ExternalInput").ap()
    yT_d = nc.dram_tensor("yT", [D, S], f32, kind="ExternalOutput").ap()
    if pat_width:
        mk_d = nc.dram_tensor("masks", [128, pat_width], f16,
                              kind="ExternalInput").ap()

    pat_off = []
    off = 0
    for p in patterns:
        pat_off.append(off)
        off += p.shape[1]

    with tile.TileContext(nc) as tc, ExitStack() as ctx:
        cpool = ctx.enter_context(tc.tile_pool(name="cpool", bufs=1))
        qT = cpool.tile([128, DT, S], f16)
        kT2 = cpool.tile([128, DT, HPM, S], f16)
        vE = cpool.tile([128, ST, H * 65 + 63], f16)
        ao = [cpool.tile([128, S], f16, name=f"ao{m}")
              for m in range(DT)]
        bvb = cpool.tile([128, H, DK], f32)
        bqs = cpool.tile([128, DT], f32)
        bks = cpool.tile([128, DT], f32)
        bos = cpool.tile([128, DT], f32)

        # big input loads: batched single DMAs on the (otherwise idle)
        # GpSimd SWDGE ring; everything else on the SP HWDGE ring
        xt = cpool.tile([128, DT, S], f16)
        wvt = cpool.tile([128, DT, D], f16)
        wqt = cpool.tile([128, DT, D], f16)
        wkt = cpool.tile([128, DT, D], f16)
        wot = cpool.tile([128, DT, D], f16)
        xT_r = xT_d.rearrange("(k p) s -> p k s", p=128)
        wv_r = wv_d.rearrange("(k p) n -> p k n", p=128)
        nc.sync.dma_start(out=bqs, in_=bq_d)
        nc.sync.dma_start(out=bks, in_=bk_d)
        nc.sync.dma_start(out=bos, in_=bo_d)
        nc.sync.dma_start(out=bvb, in_=bv_d)
        if pat_width:
            mks = cpool.tile([128, pat_width], f16)
            nc.sync.dma_start(out=mks, in_=mk_d)
        for k in range(DT):
            nc.gpsimd.dma_start(out=xt[:, k, :], in_=xT_r[:, k, :])
        nc.sync.dma_start(out=wvt[:, 0:2, :], in_=wv_r[:, 0:2, :])
        nc.sync.dma_start(out=wvt[:, 2:4, :], in_=wv_r[:, 2:4, :])
        nc.sync.dma_start(out=wvt[:, 4:6, :], in_=wv_r[:, 4:6, :])
        nc.scalar.dma_start(out=wqt, in_=wq_d.rearrange("(k p) n -> p k n",
                                                        p=128))
        nc.gpsimd.dma_start(out=wkt, in_=wk_d.rearrange("(k p) n -> p k n",
                                                        p=128))
        nc.gpsimd.dma_start(out=wot, in_=wo_d.rearrange("(k p) n -> p k n",
                                                        p=128))
        for st in range(ST):
            ve_h = vE[:, st, 0:H * 65].rearrange("p (h e) -> p h e", e=65)
            nc.vector.memset(ve_h[:, :, DK:DK + 1], 1.0)
        nc.vector.memset(vE[:, :, H * 65:], 0.0)
        nc.vector.memset(kT2[64:128, :, 0, :], 0.0)
        nc.vector.memset(kT2[0:64, :, 1, :], 0.0)

        ps_pj = ctx.enter_context(
            tc.tile_pool(name="ps_pj", bufs=2, space="PSUM"))
        ps_s = ctx.enter_context(
            tc.tile_pool(name="ps_s", bufs=4, space="PSUM"))
        ps_o = ctx.enter_context(
            tc.tile_pool(name="ps_o", bufs=2, space="PSUM"))
        apool = ctx.enter_context(tc.tile_pool(name="apool", bufs=1))
        dpool = ctx.enter_context(
            tc.tile_pool(name="dpool", bufs=1, space="DRAM"))

        # ---- V projection (natural layout) ----
        for st in range(ST):
            for h0, w in ((0, 512), (8, 256)):
                pv = ps_pj.tile([128, 512], f32, tag="pj", name="pv")
                for k in range(DT):
                    nc.tensor.matmul(
                        pv[:, :w],
                        xt[:, k, st * 128:(st + 1) * 128],
                        wvt[:, k, h0 * DK:h0 * DK + w],
                        start=(k == 0), stop=(k == DT - 1))
                nh = w // DK
                ve_h = vE[:, st, h0 * 65:(h0 + nh) * 65].rearrange(
                    "p (h e) -> p h e", e=65)
                nc.vector.tensor_add(
                    ve_h[:, :, 0:DK],
                    pv[:, :w].rearrange("p (h d) -> p h d", d=DK),
                    bvb[:, h0:h0 + nh, :])

        # ---- per m-tile: q/k projections, then attention for 2 heads ----
        rsm = None
        for m in range(DT):
            if m % 2 == 0:
                rsm = apool.tile([8, 512], f32, tag="rsm", bufs=2,
                                 name="rsm")
                pair_pos = []
            for wt, bt in ((wqt, bqs), (wkt, bks)):
                for nb in range(NB):
                    pq = ps_pj.tile([128, 512], f32, tag="pj", name="pq")
                    for k in range(DT):
                        nc.tensor.matmul(
                            pq, wt[:, k, m * 128:(m + 1) * 128],
                            xt[:, k, nb * 512:(nb + 1) * 512],
                            start=(k == 0), stop=(k == DT - 1))
                    cols = slice(nb * 512, (nb + 1) * 512)
                    if wt is wqt:
                        nc.vector.tensor_scalar_add(
                            qT[:, m, cols], pq, bt[:, m:m + 1])
                    else:
                        nc.vector.tensor_scalar_add(
                            kT2[0:64, m, 0, cols], pq[0:64, :],
                            bt[0:64, m:m + 1])
                        nc.vector.tensor_scalar_add(
                            kT2[64:128, m, 1, cols], pq[64:128, :],
                            bt[64:128, m:m + 1])

            for hh in range(HPM):
                h = m * HPM + hh
                for j in range(NB):
                    needed = [i for i in range(ST)
                              if blocks[j][i] is not None]
                    po = ps_o.tile([128, 512], f32, tag="po", name="po")
                    ets = []
                    for i in needed:
                        d = blocks[j][i]
                        lo = d["exp_lo"]
                        pss = ps_s.tile([128, 512], f32, tag="ps",
                                        name="pss")
                        nc.tensor.matmul(
                            pss[:, lo:512],
                            kT2[:, m, hh, i * 128:(i + 1) * 128],
                            qT[:, m, j * 512 + lo:(j + 1) * 512],
                            start=True, stop=True)
                        et = apool.tile([128, 512], f16, tag="et",
                                        bufs=8, name="et")
                        nc.scalar.activation(out=et[:, lo:512],
                                             in_=pss[:, lo:512],
                                             func=AF.Exp)
                        if d["mul"] is not None:
                            mlo, mhi = d["mul"]
                            poff = pat_off[d["pat_id"]]
                            nc.vector.tensor_mul(
                                et[:, mlo:mhi], et[:, mlo:mhi],
                                mks[:, poff:poff + (mhi - mlo)])
                        ets.append((i, lo, et))
                    for n, (i, lo, et) in enumerate(ets):
                        nc.tensor.matmul(
                            po[:, lo:512],
                            vE[:, i, h * 65:h * 65 + 128], et[:, lo:512],
                            start=(n == 0), stop=(n == len(ets) - 1))
                    # collect softmax denominator row; stash the
                    # unnormalized outU^T (normalized in-place per m-tile).
                    # PSUM is not DMA-able, so hop through SBUF via an ACT
                    # copy at the same partition, then row-DMA into rsm.
                    r = 4 * (m % 2) + 2 * hh + j
                    tmq = apool.tile([DK + 1, 512], f32, tag="tmq", bufs=2,
                                     name="tmq")
                    nc.scalar.activation(out=tmq[DK:DK + 1, :],
                                         in_=po[DK:DK + 1, :], func=AF.Copy)
                    nc.sync.dma_start(out=rsm[r:r + 1, :],
                                      in_=tmq[DK:DK + 1, :])
                    if hh == 0:
                        nc.vector.tensor_copy(
                            ao[m][0:DK, j * 512:(j + 1) * 512], po[0:DK, :])
                    else:
                        nc.vector.tensor_copy(
                            stg[:, j * 512:(j + 1) * 512], po[0:DK, :])
                if hh == 1:
                    nc.gpsimd.dma_start(out=ao[m][DK:128, :], in_=stg)
                else:
                    stg = apool.tile([DK, S], f16, tag="stg", bufs=2,
                                     name="stg")
            if m % 2 == 0:
                continue
            # batched normalization for this m-pair (4 heads x 2 j-blocks)
            rrm = apool.tile([8, 512], f32, tag="rrm", bufs=2, name="rrm")
            nc.vector.reciprocal(rrm, rsm)
            scr = dpool.tile([8, 512], f32, tag="scr", bufs=2, name="scr")
            nc.sync.dma_start(out=scr, in_=rrm)
            for mm in (m - 1, m):
                rt = apool.tile([128, S], f32, tag="rt", bufs=2, name="rt")
                base = scr.offset + 4 * (mm % 2) * 512
                bc0 = bass.AP(tensor=scr.tensor, offset=base,
                              ap=[[0, DK], [1, S]])
                bc1 = bass.AP(tensor=scr.tensor, offset=base + S,
                              ap=[[0, DK], [1, S]])
                nc.sync.dma_start(out=rt[0:DK, :], in_=bc0)
                nc.sync.dma_start(out=rt[DK:128, :], in_=bc1)
                nc.vector.tensor_mul(ao[mm], ao[mm], rt)

        # ---- output projection ----
        for m in range(DT):
            for nb in range(NB):
                py = ps_pj.tile([128, 512], f32, tag="pj", name="py")
                for k in range(DT):
                    nc.tensor.matmul(
                        py, wot[:, k, m * 128:(m + 1) * 128],
                        ao[k][:, nb * 512:(nb + 1) * 512],
                        start=(k == 0), stop=(k == DT - 1))
                yt = apool.tile([128, 512], f32, tag="yt", bufs=3,
                                name="yt")
                nc.scalar.activation(out=yt, in_=py, func=AF.Identity,
                                     bias=bos[:, m:m + 1])
                nc.scalar.dma_start(
                    out=yT_d[m * 128:(m + 1) * 128,
                             nb * 512:(nb + 1) * 512],
                    in_=yt)

    nc.compile()
    return nc


def prepare(x, mask, W_q, b_q, W_k, b_k, W_v, b_v, W_o, b_o):
    """Compile (cached) and build per-core input maps."""
    x = np.asarray(x, np.float32)
    mask_b = np.asarray(mask).reshape(S, S) != 0
    blocks, patterns = _classify_mask(mask_b)
    key = mask_b.tobytes()
    if key not in _CACHE:
        pat_width = sum(p.shape[1] for p in patterns)
        _CACHE[key] = (_build(blocks, patterns, pat_width), patterns)
    nc, patterns = _CACHE[key]

    xT = np.ascontiguousarray(x.transpose(0, 2, 1))          # [B, D, S]
    base = {
        "wq": np.ascontiguousarray(
            (np.asarray(W_q, np.float32) / np.sqrt(DK)).astype(np.float16)),
        "wk": np.ascontiguousarray(np.asarray(W_k, np.float16)),
        "wv": np.ascontiguousarray(np.asarray(W_v, np.float16)),
        "wo": np.ascontiguousarray(np.asarray(W_o, np.float16)),
        "bq": np.ascontiguousarray(
            (np.asarray(b_q, np.float32) / np.sqrt(DK)).reshape(DT, 128).T),
        "bk": np.ascontiguousarray(
            np.asarray(b_k, np.float32).reshape(DT, 128).T),
        "bv": np.ascontiguousarray(np.broadcast_to(
            np.asarray(b_v, np.float32).reshape(1, H, DK), (128, H, DK))),
        "bo": np.ascontiguousarray(
            np.asarray(b_o, np.float32).reshape(DT, 128).T),
    }
    if patterns:
        base["masks"] = np.ascontiguousarray(np.concatenate(patterns, axis=1))
    in_maps = [dict(base, xT=np.ascontiguousarray(xT[c]).astype(np.float16))
               for c in range(B)]
    return nc, in_maps


def kernel(**inputs):
    from concourse.bass_utils import run_bass_kernel_spmd

    nc, in_maps = prepare(**inputs)
    res = run_bass_kernel_spmd(nc, in_maps, core_ids=list(range(B)))
    out = np.stack([res.results[c]["yT"].T for c in range(B)], axis=0)
    return np.ascontiguousarray(out.astype(np.float32))
